# revision 1
# baseline (speedup 1.0000x reference)
"""GCN sampling (NodeFlow) kernel for 8 TRN2 NeuronCores.

Geometry (hardcoded per problem spec):
  N0=409600 nodes x 512 feats, layer0: 40960 dst x fanout 10, W1 [512,256]+relu,
  layer1: 4096 dst x fanout 10, W2 [256,64].

Strategy: shard layer-1 dst nodes across 8 cores (512 each). Each core pulls,
for each of its 5120 layer-1 edges, the 10 layer-0 feature rows of that edge's
src h-row (indices precomputed on host; h-rows deliberately duplicated per
layer-1 edge so BOTH segment-means become fixed-stride pooling, no on-device
gather for layer 1 and no cross-core communication). Per core:
  indirect-DMA gather 51200 rows (100 MiB) -> pool(10) -> [5120,512]
  -> PE transpose -> W1 matmul + relu -> [256hid x 5120] (hid on partitions)
  -> pool(10) along free dim -> [256 x 512] -> W2 matmul -> [512, 64].
1/10 mean factors are folded into W1, W2 on the host.
"""

import sys

sys.path.insert(0, "/opt/trn_rl_repo")

from contextlib import ExitStack

import numpy as np

N0, N1, N2 = 409600, 40960, 4096
F = 10                      # fanout
IN_F, HID, NCLS = 512, 256, 64
NC_N = 8                    # cores
DST_PC = N2 // NC_N         # 512 dst nodes per core
GRP_PC = DST_PC * F         # 5120 h-rows (groups) per core
BLK = 128                   # groups per gather block (partition dim)
NBLK = GRP_PC // BLK        # 40 blocks
SB = 4                      # blocks per matmul superblock (512 rows)
NSB = NBLK // SB            # 10 superblocks

_BUILT = None


def _legalize_waits(bir: bytes) -> bytes:
    """This container's walrus supports exactly ONE sync-wait per instruction.
    Split every multi-wait instruction: keep the last wait, hoist the others
    onto single-wait EventSemaphore instructions inserted just before it on
    the same engine (same semantics: engine sequencer blocks in order)."""
    import orjson

    j = orjson.loads(bir)
    n_new = 0
    for fn in j["functions"]:
        for bb in fn["blocks"]:
            insts = bb["instructions"]
            out = []
            for inst in insts:
                si = inst.get("sync_info")
                waits = si.get("on_wait") if si else None
                if waits and len(waits) > 1:
                    for w in waits[:-1]:
                        n_new += 1
                        out.append({
                            "debug": inst.get("debug", 0),
                            "engine": inst["engine"],
                            "ins": [],
                            "name": f"{inst['name']}_esw{n_new}",
                            "opcode": "EventSemaphore",
                            "outs": [],
                            "sync_info": {"on_update": [], "on_wait": [w]},
                        })
                    si["on_wait"] = [waits[-1]]
                out.append(inst)
            bb["instructions"] = out
    return orjson.dumps(j)


def _install_patch():
    import concourse.bass as bass

    if getattr(bass.Bass, "_gcn_wait_patch", False):
        return
    orig = bass.Bass.to_json_bytes

    def to_json_bytes(self, *a, **kw):
        return _legalize_waits(orig(self, *a, **kw))

    bass.Bass.to_json_bytes = to_json_bytes
    bass.Bass._gcn_wait_patch = True


def build_nc():
    """Build the SPMD Bass program (identical on all cores)."""
    _install_patch()
    import concourse.bass as bass
    import concourse.tile as tile
    from concourse import mybir
    from concourse.masks import make_identity

    f32 = mybir.dt.float32
    nc = bass.Bass("TRN2", target_bir_lowering=False, debug=False,
                   num_devices=NC_N, num_swdge_queues=4)

    feat = nc.dram_tensor("feat", [N0, IN_F], f32, kind="ExternalInput")
    w1 = nc.dram_tensor("w1", [IN_F, HID], f32, kind="ExternalInput")
    b1 = nc.dram_tensor("b1", [HID], f32, kind="ExternalInput")
    w2 = nc.dram_tensor("w2", [HID, NCLS], f32, kind="ExternalInput")
    b2 = nc.dram_tensor("b2", [NCLS], f32, kind="ExternalInput")
    idx = nc.dram_tensor("idx", [BLK, NBLK * F], mybir.dt.int32,
                         kind="ExternalInput")
    out = nc.dram_tensor("out", [DST_PC, NCLS], f32, kind="ExternalOutput")

    with tile.TileContext(nc) as tc, ExitStack() as ctx:
        consts = ctx.enter_context(tc.tile_pool(name="consts", bufs=1))
        gpool = ctx.enter_context(tc.tile_pool(name="gather", bufs=3))
        tpool = ctx.enter_context(tc.tile_pool(name="temps", bufs=2))
        hpool = ctx.enter_context(tc.tile_pool(name="hhT", bufs=2))
        h1pool = ctx.enter_context(tc.tile_pool(name="hh1T", bufs=1))
        opool = ctx.enter_context(tc.tile_pool(name="outs", bufs=2))
        ps_tr = ctx.enter_context(tc.tile_pool(name="ps_tr", bufs=3, space="PSUM"))
        ps_mm = ctx.enter_context(tc.tile_pool(name="ps_mm", bufs=2, space="PSUM"))
        ps_o = ctx.enter_context(tc.tile_pool(name="ps_o", bufs=2, space="PSUM"))

        idx_t = consts.tile([BLK, NBLK * F], mybir.dt.int32)
        nc.gpsimd.dma_start(idx_t[:], idx.ap())
        w1t = consts.tile([128, 4 * HID], f32)       # col fc*256+h = w1[fc*128+p, h]
        nc.gpsimd.dma_start(w1t[:].rearrange("p (f h) -> p f h", f=4),
                            w1.ap().rearrange("(f p) h -> p f h", f=4))
        w2t = consts.tile([128, 2 * NCLS], f32)      # col c*64+n = w2[c*128+p, n]
        nc.gpsimd.dma_start(w2t[:].rearrange("p (c n) -> p c n", c=2),
                            w2.ap().rearrange("(c p) n -> p c n", c=2))
        b1t = consts.tile([128, 2], f32)             # col h = b1[h*128+p]
        nc.gpsimd.dma_start(b1t[:], b1.ap().rearrange("(h p) -> p h", h=2))
        b2t = consts.tile([1, NCLS], f32)
        nc.gpsimd.dma_start(b2t[:], b2.ap().unsqueeze(0))
        ident = consts.tile([128, 128], f32)
        make_identity(nc, ident[:])

        # hh1T[hc]: [128 hid, 5120 rows], rows = layer-1 edges
        hh1T = [h1pool.tile([128, GRP_PC], f32, tag=f"hh1T{hc}",
                               name=f"hh1T{hc}") for hc in range(2)]

        hhT_sb = None
        for b in range(NBLK):
            if b % SB == 0:
                hhT_sb = [hpool.tile([128, SB * 128], f32, tag=f"hhT{fc}",
                                     name=f"hhT{fc}_{b}") for fc in range(4)]
            # gather 128 groups x 512 feats per edge-slot k (HW indirect DMA
            # honors exactly one index per partition per instruction)
            g = []
            for k in range(F):
                gk = gpool.tile([BLK, IN_F], mybir.dt.float32, tag=f"g{k}",
                                name=f"g{k}_{b}")
                gi = nc.gpsimd.indirect_dma_start(
                    out=gk[:], out_offset=None, in_=feat.ap(),
                    in_offset=bass.IndirectOffsetOnAxis(
                        ap=idx_t[:, b * F + k:b * F + k + 1], axis=0),
                )
                q = (b * F + k) % 4
                if q:  # spread gathers over the 4 SWDGE queues
                    gi.ins.queue = f"qPoolDynamic{q}"
                g.append(gk)
            # pool the 10 edges (tree adds on DVE)
            s0 = tpool.tile([BLK, IN_F], f32, tag="s0")
            s1 = tpool.tile([BLK, IN_F], f32, tag="s1")
            s2 = tpool.tile([BLK, IN_F], f32, tag="s2")
            s3 = tpool.tile([BLK, IN_F], f32, tag="s3")
            s4 = tpool.tile([BLK, IN_F], f32, tag="s4")
            hs = tpool.tile([BLK, IN_F], f32, tag="hs")
            E = lambda k: g[k][:]
            nc.vector.tensor_add(s0[:], E(0), E(1))
            nc.vector.tensor_add(s1[:], E(2), E(3))
            nc.vector.tensor_add(s2[:], E(4), E(5))
            nc.vector.tensor_add(s3[:], E(6), E(7))
            nc.vector.tensor_add(s4[:], E(8), E(9))
            nc.vector.tensor_add(s0[:], s0[:], s1[:])
            nc.vector.tensor_add(s2[:], s2[:], s3[:])
            nc.vector.tensor_add(s0[:], s0[:], s2[:])
            nc.vector.tensor_add(hs[:], s0[:], s4[:])
            # transpose to [feat, rows] chunks
            col = (b % SB) * 128
            for fc in range(4):
                ptr = ps_tr.tile([128, 128], f32, tag="ptr", space="PSUM")
                nc.tensor.transpose(ptr[:], hs[:, fc * 128:(fc + 1) * 128],
                                    ident[:])
                nc.vector.tensor_copy(hhT_sb[fc][:, col:col + 128], ptr[:])
            if b % SB == SB - 1:
                sb = b // SB
                rows = slice(sb * SB * 128, (sb + 1) * SB * 128)
                for hc in range(2):
                    pm = ps_mm.tile([128, SB * 128], f32, tag="pm", space="PSUM")
                    for fc in range(4):
                        nc.tensor.matmul(
                            pm[:],
                            lhsT=w1t[:, fc * HID + hc * 128: fc * HID + hc * 128 + 128],
                            rhs=hhT_sb[fc][:],
                            start=(fc == 0), stop=(fc == 3),
                        )
                    nc.scalar.activation(hh1T[hc][:, rows], pm[:],
                                         mybir.ActivationFunctionType.Relu,
                                         bias=b1t[:, hc:hc + 1])

        # layer-1 pooling along free dim: [128, 5120] -> [128, 512]
        g2 = []
        for hc in range(2):
            v = hh1T[hc][:].rearrange("p (r k) -> p r k", k=F)
            t0 = tpool.tile([128, DST_PC], f32, tag="p2a", bufs=1)
            t1 = tpool.tile([128, DST_PC], f32, tag="p2b", bufs=1)
            t2 = tpool.tile([128, DST_PC], f32, tag="p2c", bufs=1)
            t3 = tpool.tile([128, DST_PC], f32, tag="p2d", bufs=1)
            t4 = tpool.tile([128, DST_PC], f32, tag="p2e", bufs=1)
            gg = tpool.tile([128, DST_PC], f32, tag=f"g2_{hc}", bufs=1)
            V = lambda k: v[:, :, k]
            nc.vector.tensor_add(t0[:], V(0), V(1))
            nc.vector.tensor_add(t1[:], V(2), V(3))
            nc.vector.tensor_add(t2[:], V(4), V(5))
            nc.vector.tensor_add(t3[:], V(6), V(7))
            nc.vector.tensor_add(t4[:], V(8), V(9))
            nc.vector.tensor_add(t0[:], t0[:], t1[:])
            nc.vector.tensor_add(t2[:], t2[:], t3[:])
            nc.vector.tensor_add(t0[:], t0[:], t2[:])
            nc.vector.tensor_add(gg[:], t0[:], t4[:])
            g2.append(gg)

        # final matmul: out[r, n] = sum_h g2[h, r] * w2[h, n] + b2
        for rb in range(DST_PC // 128):
            po = ps_o.tile([128, NCLS], f32, tag="po", space="PSUM")
            for hc in range(2):
                nc.tensor.matmul(
                    po[:],
                    lhsT=g2[hc][:, rb * 128:(rb + 1) * 128],
                    rhs=w2t[:, hc * NCLS:(hc + 1) * NCLS],
                    start=(hc == 0), stop=(hc == 1),
                )
            ot = opool.tile([128, NCLS], f32, tag="ot")
            nc.vector.tensor_copy(ot[:], po[:])
            nc.gpsimd.dma_start(out.ap()[rb * 128:(rb + 1) * 128, :], ot[:])

    return nc


def _get_nc():
    global _BUILT
    if _BUILT is None:
        _BUILT = build_nc()
    return _BUILT


def _prep_core_indices(src0, src1, core):
    s1 = src1[core * GRP_PC:(core + 1) * GRP_PC].astype(np.int64)
    G = src0[(s1[:, None] * F + np.arange(F)[None, :])]        # [5120, 10]
    return np.ascontiguousarray(
        G.reshape(NBLK, BLK, F).transpose(1, 0, 2).reshape(BLK, NBLK * F)
    ).astype(np.int32)


def _run(inputs, trace=False, trace_kwargs=None):
    from concourse.bass_utils import run_bass_kernel_spmd

    features = np.ascontiguousarray(inputs["features"], dtype=np.float32)
    w1s = np.ascontiguousarray(inputs["W1"], dtype=np.float32) / np.float32(F)
    w2s = np.ascontiguousarray(inputs["W2"], dtype=np.float32) / np.float32(F)
    b1 = np.ascontiguousarray(inputs["b1"], dtype=np.float32)
    b2 = np.ascontiguousarray(inputs["b2"], dtype=np.float32)
    src0 = np.asarray(inputs["src0"]).astype(np.int64)
    src1 = np.asarray(inputs["src1"]).astype(np.int64)

    in_maps = []
    for c in range(NC_N):
        in_maps.append({
            "feat": features, "w1": w1s, "b1": b1, "w2": w2s, "b2": b2,
            "idx": _prep_core_indices(src0, src1, c),
        })
    nc = _get_nc()
    kw = {}
    if trace:
        kw = {"trace": True, "trace_kwargs": trace_kwargs or {}}
    res = run_bass_kernel_spmd(nc, in_maps, list(range(NC_N)), **kw)
    full = np.concatenate([res.results[c]["out"] for c in range(NC_N)], axis=0)
    full = full + b2[None, :]
    return full, res


def kernel(features, W1, b1, W2, b2, src0, dst0, src1, dst1):
    ins = dict(features=features, W1=W1, b1=b1, W2=W2, b2=b2,
               src0=src0, dst0=dst0, src1=src1, dst1=dst1)
    d0 = np.asarray(dst0); d1 = np.asarray(dst1)
    fixed = (d0 == np.arange(N1 * F) // F).all() and \
            (d1 == np.arange(N2 * F) // F).all()
    if not fixed:
        # general (unexpected) dst pattern: numpy fallback for correctness
        f = np.asarray(features, dtype=np.float64)
        m = f[np.asarray(src0)]
        s = np.zeros((N1, IN_F)); np.add.at(s, d0, m)
        deg = np.bincount(d0, minlength=N1).clip(1)
        h = np.maximum(s / deg[:, None] @ np.asarray(W1) + np.asarray(b1), 0)
        m = h[np.asarray(src1)]
        s = np.zeros((N2, HID)); np.add.at(s, d1, m)
        deg = np.bincount(d1, minlength=N2).clip(1)
        return ((s / deg[:, None]) @ np.asarray(W2) + np.asarray(b2)
                ).astype(np.float32)
    out, _ = _run(ins)
    return out



# revision 2
# speedup vs baseline: 1.0229x; 1.0229x over previous
"""GCN sampling kernel v2: bf16 features + accumulate-in-DMA pooling.

Same sharding as baseline (512 layer-1 dst per core, h-rows duplicated per
layer-1 edge so both segment-means are fixed-stride). Changes vs baseline:
  - features cast to bf16 on host: halves gather bytes (DMA 291->146us)
  - layer-0 pooling done BY THE DMA (compute_op=add accumulate-gather):
    k=0 gather writes, k=1..9 accumulate. Eliminates all DVE pooling adds.
  - transposes via regular matmul against identity (bf16 in, f32 PSUM out)
  - all non-gather DMAs moved off the Pool engine (sync/scalar HWDGE)
  - weights bf16, 1/10 mean factors folded into W1, W2 on host
"""

import sys

sys.path.insert(0, "/opt/trn_rl_repo")

from contextlib import ExitStack

import numpy as np
import ml_dtypes

N0, N1, N2 = 409600, 40960, 4096
F = 10
IN_F, HID, NCLS = 512, 256, 64
NC_N = 8
DST_PC = N2 // NC_N         # 512 dst nodes per core
GRP_PC = DST_PC * F         # 5120 h-rows (groups) per core
BLK = 128
NBLK = GRP_PC // BLK        # 40 blocks
SB = 4                      # blocks per matmul superblock
NSB = NBLK // SB

_BUILT = None


def _legalize_waits(bir: bytes) -> bytes:
    """Split multi-wait instructions (walrus here supports one wait each)."""
    import orjson

    j = orjson.loads(bir)
    n_new = 0
    for fn in j["functions"]:
        for bb in fn["blocks"]:
            insts = bb["instructions"]
            out = []
            for inst in insts:
                si = inst.get("sync_info")
                waits = si.get("on_wait") if si else None
                if waits and len(waits) > 1:
                    for w in waits[:-1]:
                        n_new += 1
                        out.append({
                            "debug": inst.get("debug", 0),
                            "engine": inst["engine"],
                            "ins": [],
                            "name": f"{inst['name']}_esw{n_new}",
                            "opcode": "EventSemaphore",
                            "outs": [],
                            "sync_info": {"on_update": [], "on_wait": [w]},
                        })
                    si["on_wait"] = [waits[-1]]
                out.append(inst)
            bb["instructions"] = out
    return orjson.dumps(j)


def _install_patch():
    import concourse.bass as bass

    if getattr(bass.Bass, "_gcn_wait_patch", False):
        return
    orig = bass.Bass.to_json_bytes

    def to_json_bytes(self, *a, **kw):
        return _legalize_waits(orig(self, *a, **kw))

    bass.Bass.to_json_bytes = to_json_bytes
    bass.Bass._gcn_wait_patch = True


def build_nc():
    _install_patch()
    import concourse.bass as bass
    import concourse.tile as tile
    from concourse import mybir
    from concourse.masks import make_identity

    f32 = mybir.dt.float32
    bf16 = mybir.dt.bfloat16
    nc = bass.Bass("TRN2", target_bir_lowering=False, debug=False,
                   num_devices=NC_N, num_swdge_queues=4)

    feat = nc.dram_tensor("feat", [N0, IN_F], bf16, kind="ExternalInput")
    w1 = nc.dram_tensor("w1", [IN_F, HID], bf16, kind="ExternalInput")
    b1 = nc.dram_tensor("b1", [HID], f32, kind="ExternalInput")
    w2 = nc.dram_tensor("w2", [HID, NCLS], bf16, kind="ExternalInput")
    idx = nc.dram_tensor("idx", [BLK, NBLK * F], mybir.dt.int32,
                         kind="ExternalInput")
    out = nc.dram_tensor("out", [DST_PC, NCLS], f32, kind="ExternalOutput")

    with tile.TileContext(nc) as tc, ExitStack() as ctx:
        consts = ctx.enter_context(tc.tile_pool(name="consts", bufs=1))
        gpool = ctx.enter_context(tc.tile_pool(name="gather", bufs=6))
        hpool = ctx.enter_context(tc.tile_pool(name="hhT", bufs=2))
        h1pool = ctx.enter_context(tc.tile_pool(name="hh1T", bufs=1))
        tpool = ctx.enter_context(tc.tile_pool(name="temps", bufs=2))
        opool = ctx.enter_context(tc.tile_pool(name="outs", bufs=2))
        ps_tr = ctx.enter_context(tc.tile_pool(name="ps_tr", bufs=4, space="PSUM"))
        ps_mm = ctx.enter_context(tc.tile_pool(name="ps_mm", bufs=2, space="PSUM"))
        ps_o = ctx.enter_context(tc.tile_pool(name="ps_o", bufs=2, space="PSUM"))

        idx_t = consts.tile([BLK, NBLK * F], mybir.dt.int32)
        nc.sync.dma_start(idx_t[:], idx.ap())
        # w1t col fc*256+h = w1[fc*128+p, h]
        w1t = consts.tile([128, 4 * HID], bf16)
        nc.sync.dma_start(w1t[:].rearrange("p (f h) -> p f h", f=4),
                          w1.ap().rearrange("(f p) h -> p f h", f=4))
        w2t = consts.tile([128, 2 * NCLS], bf16)
        nc.sync.dma_start(w2t[:].rearrange("p (c n) -> p c n", c=2),
                          w2.ap().rearrange("(c p) n -> p c n", c=2))
        b1t = consts.tile([128, 2], f32)
        nc.sync.dma_start(b1t[:], b1.ap().rearrange("(h p) -> p h", h=2))
        ident = consts.tile([128, 128], bf16)
        make_identity(nc, ident[:])

        # hh1T[hc]: [128 hid, 5120 groups] bf16
        hh1T = [h1pool.tile([128, GRP_PC], bf16, tag=f"hh1T{hc}",
                            name=f"hh1T{hc}") for hc in range(2)]

        hhT_sb = None
        for b in range(NBLK):
            if b % SB == 0:
                hhT_sb = [hpool.tile([128, SB * 128], bf16, tag=f"hhT{fc}",
                                     name=f"hhT{fc}_{b}") for fc in range(4)]
            # pooled[g, f] accumulated by the DMA: k=0 writes, k>0 adds
            hs = gpool.tile([BLK, IN_F], bf16, tag="hs", name=f"hs_{b}")
            for k in range(F):
                nc.gpsimd.indirect_dma_start(
                    out=hs[:], out_offset=None, in_=feat.ap(),
                    in_offset=bass.IndirectOffsetOnAxis(
                        ap=idx_t[:, b * F + k:b * F + k + 1], axis=0),
                    compute_op=(mybir.AluOpType.bypass if k == 0
                                else mybir.AluOpType.add),
                )
            # transpose via matmul: out[f, g] = sum_g' hs[g', f] ident[g', g]
            col = (b % SB) * 128
            for fc in range(4):
                ptr = ps_tr.tile([128, 128], f32, tag="ptr", space="PSUM")
                nc.tensor.matmul(ptr[:], lhsT=hs[:, fc * 128:(fc + 1) * 128],
                                 rhs=ident[:], start=True, stop=True)
                nc.vector.tensor_copy(hhT_sb[fc][:, col:col + 128], ptr[:])
            if b % SB == SB - 1:
                sb = b // SB
                rows = slice(sb * SB * 128, (sb + 1) * SB * 128)
                for hc in range(2):
                    pm = ps_mm.tile([128, SB * 128], f32, tag="pm",
                                    space="PSUM")
                    for fc in range(4):
                        nc.tensor.matmul(
                            pm[:],
                            lhsT=w1t[:, fc * HID + hc * 128:
                                     fc * HID + hc * 128 + 128],
                            rhs=hhT_sb[fc][:],
                            start=(fc == 0), stop=(fc == 3),
                        )
                    nc.scalar.activation(hh1T[hc][:, rows], pm[:],
                                         mybir.ActivationFunctionType.Relu,
                                         bias=b1t[:, hc:hc + 1])

        # layer-1 pooling along free dim: [128, 5120] -> [128, 512]
        g2 = []
        for hc in range(2):
            v = hh1T[hc][:].rearrange("p (r k) -> p r k", k=F)
            t0 = tpool.tile([128, DST_PC], bf16, tag="p2a", bufs=1)
            t1 = tpool.tile([128, DST_PC], bf16, tag="p2b", bufs=1)
            t2 = tpool.tile([128, DST_PC], bf16, tag="p2c", bufs=1)
            t3 = tpool.tile([128, DST_PC], bf16, tag="p2d", bufs=1)
            t4 = tpool.tile([128, DST_PC], bf16, tag="p2e", bufs=1)
            gg = tpool.tile([128, DST_PC], bf16, tag=f"g2_{hc}", bufs=1)
            V = lambda k: v[:, :, k]
            nc.vector.tensor_add(t0[:], V(0), V(1))
            nc.vector.tensor_add(t1[:], V(2), V(3))
            nc.vector.tensor_add(t2[:], V(4), V(5))
            nc.vector.tensor_add(t3[:], V(6), V(7))
            nc.vector.tensor_add(t4[:], V(8), V(9))
            nc.vector.tensor_add(t0[:], t0[:], t1[:])
            nc.vector.tensor_add(t2[:], t2[:], t3[:])
            nc.vector.tensor_add(t0[:], t0[:], t2[:])
            nc.vector.tensor_add(gg[:], t0[:], t4[:])
            g2.append(gg)

        # final matmul: out[r, n] = sum_h g2[h, r] * w2[h, n]
        for rb in range(DST_PC // 128):
            po = ps_o.tile([128, NCLS], f32, tag="po", space="PSUM")
            for hc in range(2):
                nc.tensor.matmul(
                    po[:],
                    lhsT=g2[hc][:, rb * 128:(rb + 1) * 128],
                    rhs=w2t[:, hc * NCLS:(hc + 1) * NCLS],
                    start=(hc == 0), stop=(hc == 1),
                )
            ot = opool.tile([128, NCLS], f32, tag="ot")
            nc.vector.tensor_copy(ot[:], po[:])
            nc.sync.dma_start(out.ap()[rb * 128:(rb + 1) * 128, :], ot[:])

    return nc


def _get_nc():
    global _BUILT
    if _BUILT is None:
        _BUILT = build_nc()
    return _BUILT


def _prep_core_indices(src0, src1, core):
    s1 = src1[core * GRP_PC:(core + 1) * GRP_PC].astype(np.int64)
    G = src0[(s1[:, None] * F + np.arange(F)[None, :])]        # [5120, 10]
    return np.ascontiguousarray(
        G.reshape(NBLK, BLK, F).transpose(1, 0, 2).reshape(BLK, NBLK * F)
    ).astype(np.int32)


def _run(inputs, trace=False, trace_kwargs=None):
    from concourse.bass_utils import run_bass_kernel_spmd

    featb = np.ascontiguousarray(
        np.asarray(inputs["features"], dtype=np.float32)
    ).astype(ml_dtypes.bfloat16)
    w1s = (np.ascontiguousarray(inputs["W1"], dtype=np.float32)
           / np.float32(F)).astype(ml_dtypes.bfloat16)
    w2s = (np.ascontiguousarray(inputs["W2"], dtype=np.float32)
           / np.float32(F)).astype(ml_dtypes.bfloat16)
    b1 = np.ascontiguousarray(inputs["b1"], dtype=np.float32)
    b2 = np.ascontiguousarray(inputs["b2"], dtype=np.float32)
    src0 = np.asarray(inputs["src0"]).astype(np.int64)
    src1 = np.asarray(inputs["src1"]).astype(np.int64)

    in_maps = []
    for c in range(NC_N):
        in_maps.append({
            "feat": featb, "w1": w1s, "b1": b1, "w2": w2s,
            "idx": _prep_core_indices(src0, src1, c),
        })
    nc = _get_nc()
    kw = {}
    if trace:
        kw = {"trace": True, "trace_kwargs": trace_kwargs or {}}
    res = run_bass_kernel_spmd(nc, in_maps, list(range(NC_N)), **kw)
    full = np.concatenate([res.results[c]["out"] for c in range(NC_N)], axis=0)
    full = full + b2[None, :]
    return full, res


def kernel(features, W1, b1, W2, b2, src0, dst0, src1, dst1):
    ins = dict(features=features, W1=W1, b1=b1, W2=W2, b2=b2,
               src0=src0, dst0=dst0, src1=src1, dst1=dst1)
    d0 = np.asarray(dst0); d1 = np.asarray(dst1)
    fixed = (d0 == np.arange(N1 * F) // F).all() and \
            (d1 == np.arange(N2 * F) // F).all()
    if not fixed:
        f = np.asarray(features, dtype=np.float64)
        m = f[np.asarray(src0)]
        s = np.zeros((N1, IN_F)); np.add.at(s, d0, m)
        deg = np.bincount(d0, minlength=N1).clip(1)
        h = np.maximum(s / deg[:, None] @ np.asarray(W1) + np.asarray(b1), 0)
        m = h[np.asarray(src1)]
        s = np.zeros((N2, HID)); np.add.at(s, d1, m)
        deg = np.bincount(d1, minlength=N2).clip(1)
        return ((s / deg[:, None]) @ np.asarray(W2) + np.asarray(b2)
                ).astype(np.float32)
    out, _ = _run(ins)
    return out


if __name__ == "__main__":
    # quick TimelineSim estimate
    from concourse.timeline_sim import TimelineSim
    ts = TimelineSim(_get_nc(), trace=False)
    ts.simulate()
    print("TimelineSim:", int(ts.time), "ns")
